# revision 26
# baseline (speedup 1.0000x reference)
"""
AdaptPooling (seed-attention segment softmax pooling) on 8 TRN2 NeuronCores.

Math (reference):
    key   = x @ W_key + b_key            [N, H, C]
    alpha = <key, seed> / sqrt(C)        [N, H]
    softmax of alpha within each segment (batch is SORTED)
    out   = segsum(alpha_n * (x @ W_value + b_value))    [B, H*C]
    new_position = segmean(position)                     [B, 3]
    new_edge_index = concat(batch[raw_edge_index], swap) [2, 2E]

Device strategy (data-parallel over nodes, split at segment boundaries so
segment reductions never cross cores -> no collectives needed):

  * alpha only touches x through Wq[d,h] = sum_c W_key[d,h*C+c]*seed[h,c],
    so alpha = (x @ Wq + bq)/sqrt(C): a [D,H] projection (host-folded).
  * softmax max-subtraction is dropped (shift invariant; scores are O(1));
    per-head constant exp(bq/sqrt(C)) folds into the one-hot build.
  * weighted aggregation uses the moment trick:
        G[d, h, s] = sum_n x[n,d] * ex[n,h] * onehot[n,s]
        out_pre[s, hc] = sum_d G[d,h,s] * W_value[d,hc]
    This contracts over NODES (natural x layout, no value materialization,
    x is read exactly once).
  * batch is an input -> host knows the segment structure and bakes a
    static windowed layout: nodes are packed into "super-tiles" of 1024;
    each super-tile's segments fit a window of WG segment slots. PSUM
    accumulates [128d x H*WG] over the super-tile; windows are staged to
    DRAM and the (tiny) overlap-add + divide epilogue runs on host.
  * edge remap is a pure relabel gather (batch[idx]); TRN2 has no
    efficient fine-grained gather (indirect-DMA is descriptor-bound at
    4B/descriptor), so it stays on host.
"""

import math
import os
import sys
from contextlib import ExitStack

import numpy as np

sys.path.insert(0, "/opt/trn_rl_repo")

P = 128          # partitions
ST_TILES = 8     # 128-node tiles per super-tile
ST = P * ST_TILES

# set by kernel() after each run (read by test.py)
LAST_EXEC_NS = None
LAST_RESULT = None

_GRAPH_CACHE = {}


def _split_multi_waits(nc):
    """walrus on this toolchain accepts only ONE semaphore wait per
    instruction; split extra waits into preceding wait-only NoOps."""
    from concourse import mybir

    n = 0
    for fn in nc.m.functions:
        for bb in fn.blocks:
            new_list = []
            for inst in bb.instructions:
                si = inst.sync_info
                ow = list(si.on_wait) if si is not None and si.on_wait else []
                if len(ow) > 1:
                    for w in ow[:-1]:
                        nop = mybir.InstNoOp(
                            name=f"{inst.name}-wsplit{n}", ins=[], outs=[]
                        )
                        nop.engine = inst.engine
                        nop.sync_info = mybir.SyncInfo(on_wait=[w], on_update=[])
                        new_list.append(nop)
                        n += 1
                    si.on_wait = [ow[-1]]
                    inst.sync_info = si
                new_list.append(inst)
            bb.instructions = new_list
    return n


def _build_graph(NSUP, WG, D, H, C, host_alpha, kfac):
    from concourse import bass, mybir
    from concourse.tile import TileContext

    f32 = mybir.dt.float32
    HC = H * C
    DC = D // P
    AUXW = WG + 4 + (H if host_alpha else 0)
    inv_sqrt_c = 1.0 / math.sqrt(C)

    nc = bass.Bass(target_bir_lowering=False)
    x_p = nc.declare_dram_parameter("x", [NSUP * P, ST_TILES * D], f32, isOutput=False)
    aux_p = nc.declare_dram_parameter(
        "aux", [NSUP * P, ST_TILES * AUXW], f32, isOutput=False
    )
    wv_p = nc.declare_dram_parameter("wv", [D, HC], f32, isOutput=False)
    if not host_alpha:
        wq_p = nc.declare_dram_parameter("wq", [D, H], f32, isOutput=False)
    proj_p = nc.declare_dram_parameter("proj", [NSUP * C, H * WG], f32, isOutput=True)
    pd_p = nc.declare_dram_parameter("pd", [NSUP * 4, WG + H * WG], f32, isOutput=True)

    with TileContext(nc) as tc, ExitStack() as ctx:
        consts = ctx.enter_context(tc.tile_pool(name="consts", bufs=1))
        xin = ctx.enter_context(tc.tile_pool(name="xin", bufs=3))
        work = ctx.enter_context(tc.tile_pool(name="work", bufs=3))
        stg = ctx.enter_context(tc.tile_pool(name="stg", bufs=2))
        ps_g = ctx.enter_context(tc.tile_pool(name="ps_g", bufs=1, space="PSUM"))
        ps_s = ctx.enter_context(tc.tile_pool(name="ps_s", bufs=1, space="PSUM"))

        # weights, d-chunked to [128, DC, *]; laundered once via DVE
        wv_sb = consts.tile([P, DC, HC], f32)
        nc.sync.dma_start(
            out=wv_sb[:], in_=wv_p[:].rearrange("(dc p) hc -> p dc hc", p=P)
        )
        if not host_alpha:
            wq_sb = consts.tile([P, DC, H], f32)
            nc.sync.dma_start(
                out=wq_sb[:], in_=wq_p[:].rearrange("(dc p) h -> p dc h", p=P)
            )
            ident = consts.tile([P, P], f32)
            nc.gpsimd.memset(ident[:], 0.0)
            nc.gpsimd.affine_select(
                out=ident[:],
                in_=ident[:],
                compare_op=mybir.AluOpType.not_equal,
                fill=1.0,
                base=0,
                pattern=[[-1, P]],
                channel_multiplier=1,
            )

        for t in range(NSUP):
            x_t = xin.tile([P, ST_TILES * D], f32, tag="x")
            aux_t = xin.tile([P, ST_TILES * AUXW], f32, tag="aux")
            nc.sync.dma_start(out=x_t[:], in_=x_p[t * P : (t + 1) * P, :])
            nc.sync.dma_start(out=aux_t[:], in_=aux_p[t * P : (t + 1) * P, :])

            g_ps = []
            for ck in range(DC):
                g_ck = ps_g.tile([P, H * WG], f32, tag=f"g{ck}", name=f"g{ck}_{t}")
                g_ps.append(g_ck)
            pos_ps = ps_s.tile([4, WG], f32, tag="pos")
            den_ps = ps_s.tile([4, H * WG], f32, tag="den")

            for tt in range(ST_TILES):
                xa = tt * D
                aa = tt * AUXW
                cmp_ap = aux_t[:, aa : aa + WG]
                posw_ap = aux_t[:, aa + WG : aa + WG + 4]

                if host_alpha:
                    ex_cols = lambda h: aux_t[:, aa + WG + 4 + h : aa + WG + 5 + h]
                else:
                    xT_sb = []
                    for ck in range(DC):
                        xT_ps = ps_s.tile([P, P], f32, tag=f"xT{ck}")
                        nc.tensor.transpose(
                            out=xT_ps[:],
                            in_=x_t[:, xa + ck * P : xa + (ck + 1) * P],
                            identity=ident[:],
                        )
                        xs = work.tile([P, P], f32, tag=f"xTs{ck}")
                        nc.vector.tensor_copy(out=xs[:], in_=xT_ps[:])
                        xT_sb.append(xs)
                    araw_ps = ps_s.tile([P, H], f32, tag="araw")
                    for ck in range(DC):
                        nc.tensor.matmul(
                            out=araw_ps[:],
                            lhsT=xT_sb[ck][:],
                            rhs=wq_sb[:, ck, :],
                            start=(ck == 0),
                            stop=(ck == DC - 1),
                        )
                    ex_sb = work.tile([P, H], f32, tag="ex")
                    nc.scalar.activation(
                        out=ex_sb[:],
                        in_=araw_ps[:],
                        func=mybir.ActivationFunctionType.Exp,
                        scale=inv_sqrt_c,
                    )
                    ex_cols = lambda h: ex_sb[:, h : h + 1]

                oh_sb = work.tile([P, H * WG], f32, tag="oh")
                for h in range(H):
                    if abs(kfac[h] - 1.0) < 1e-12:
                        nc.vector.tensor_scalar(
                            out=oh_sb[:, h * WG : (h + 1) * WG],
                            in0=cmp_ap,
                            scalar1=ex_cols(h),
                            scalar2=None,
                            op0=mybir.AluOpType.mult,
                        )
                    else:
                        nc.vector.tensor_scalar(
                            out=oh_sb[:, h * WG : (h + 1) * WG],
                            in0=cmp_ap,
                            scalar1=ex_cols(h),
                            scalar2=float(kfac[h]),
                            op0=mybir.AluOpType.mult,
                            op1=mybir.AluOpType.mult,
                        )

                first = tt == 0
                last = tt == ST_TILES - 1
                for ck in range(DC):
                    nc.tensor.matmul(
                        out=g_ps[ck][:],
                        lhsT=x_t[:, xa + ck * P : xa + (ck + 1) * P],
                        rhs=oh_sb[:],
                        start=first,
                        stop=last,
                    )
                nc.tensor.matmul(
                    out=pos_ps[:], lhsT=posw_ap, rhs=cmp_ap, start=first, stop=last
                )
                nc.tensor.matmul(
                    out=den_ps[:], lhsT=posw_ap, rhs=oh_sb[:], start=first, stop=last
                )

            # G -> SBUF, project with W_value, stage out
            g_sb = stg.tile([P, DC, H * WG], f32, tag="gsb")
            for ck in range(DC):
                nc.vector.tensor_copy(out=g_sb[:, ck, :], in_=g_ps[ck][:])
            proj_ps = ps_s.tile([C, H * WG], f32, tag="proj")
            for h in range(H):
                for ck in range(DC):
                    nc.tensor.matmul(
                        out=proj_ps[:, h * WG : (h + 1) * WG],
                        lhsT=wv_sb[:, ck, h * C : (h + 1) * C],
                        rhs=g_sb[:, ck, h * WG : (h + 1) * WG],
                        start=(ck == 0),
                        stop=(ck == DC - 1),
                    )
            proj_sb = stg.tile([C, H * WG], f32, tag="projsb")
            nc.vector.tensor_copy(out=proj_sb[:], in_=proj_ps[:])
            pd_sb = stg.tile([4, WG + H * WG], f32, tag="pdsb")
            nc.vector.tensor_copy(out=pd_sb[:, :WG], in_=pos_ps[:])
            nc.vector.tensor_copy(out=pd_sb[:, WG:], in_=den_ps[:])
            nc.sync.dma_start(out=proj_p[t * C : (t + 1) * C, :], in_=proj_sb[:])
            nc.sync.dma_start(out=pd_p[t * 4 : (t + 1) * 4, :], in_=pd_sb[:])

    _split_multi_waits(nc)
    return nc


def _ensure_ntff_hook():
    """The image's antenv lacks axon_hooks; provide it and register the
    ctypes NTFF profile hook so trace=True yields exec_time_ns."""
    import types

    try:
        from antenv import axon_hooks  # noqa: F401
    except ImportError:
        import antenv

        m = types.ModuleType("antenv.axon_hooks")
        m._hook = None
        m.set_axon_ntff_profile_hook = lambda h: setattr(m, "_hook", h)
        m.get_axon_ntff_profile_hook = lambda: getattr(m, "_hook", None)
        sys.modules["antenv.axon_hooks"] = m
        antenv.axon_hooks = m
    from antenv.axon_hooks import (
        get_axon_ntff_profile_hook,
        set_axon_ntff_profile_hook,
    )

    if get_axon_ntff_profile_hook() is None:
        from trn_agent_boot.trn_boot import _ntff_profile_via_ctypes

        set_axon_ntff_profile_hook(
            _ntff_profile_via_ctypes("/opt/axon/libaxon_pjrt.so")
        )


class _SimRes:
    def __init__(self, results):
        self.results = results
        self.exec_time_ns = None


def _numpy_sim(in_maps, NSUP, WG, D, H, C, host_alpha, kfac):
    """Bit-faithful numpy emulation of the device graph (for layout checks)."""
    AUXW = WG + 4 + (H if host_alpha else 0)
    results = []
    for im in in_maps:
        # undo the [t, p, tt, :] interleave back to node order
        x_sl = (
            im["x"].reshape(NSUP, P, ST_TILES, D).transpose(0, 2, 1, 3).reshape(-1, D)
        )
        aux = (
            im["aux"]
            .reshape(NSUP, P, ST_TILES, AUXW)
            .transpose(0, 2, 1, 3)
            .reshape(-1, AUXW)
        )
        wv = im["wv"]
        cmp_ = aux[:, :WG]
        posw = aux[:, WG : WG + 4]
        if host_alpha:
            ex = aux[:, WG + 4 : WG + 4 + H]
        else:
            ex = np.exp((x_sl @ im["wq"]) / math.sqrt(C))
        ex = ex * np.asarray(kfac, np.float32)[None, :]
        proj = np.zeros((NSUP * C, H * WG), np.float32)
        pd = np.zeros((NSUP * 4, WG + H * WG), np.float32)
        for t in range(NSUP):
            sl = slice(t * ST, (t + 1) * ST)
            oh = (ex[sl][:, :, None] * cmp_[sl][:, None, :]).reshape(ST, H * WG)
            G = x_sl[sl].T @ oh                            # [D, H*WG]
            pos = posw[sl].T @ cmp_[sl]                    # [4, WG]
            den = posw[sl].T @ oh                          # [4, H*WG]
            pr = np.zeros((C, H * WG), np.float32)
            for h in range(H):
                pr[:, h * WG : (h + 1) * WG] = (
                    wv[:, h * C : (h + 1) * C].T @ G[:, h * WG : (h + 1) * WG]
                )
            proj[t * C : (t + 1) * C] = pr
            pd[t * 4 : (t + 1) * 4, :WG] = pos
            pd[t * 4 : (t + 1) * 4, WG:] = den
        results.append({"proj": proj, "pd": pd})
    return _SimRes(results)


def kernel(**inputs):
    global LAST_EXEC_NS, LAST_RESULT
    x = np.ascontiguousarray(np.asarray(inputs["x"], dtype=np.float32))
    position = np.ascontiguousarray(np.asarray(inputs["position"], dtype=np.float32))
    batch = np.ascontiguousarray(np.asarray(inputs["batch"], dtype=np.int32))
    rei = np.asarray(inputs["raw_edge_index"])
    B = int(np.asarray(inputs["num_segments"]))
    W_key = np.asarray(inputs["W_key"], dtype=np.float32)
    b_key = np.asarray(inputs["b_key"], dtype=np.float32)
    W_value = np.ascontiguousarray(np.asarray(inputs["W_value"], dtype=np.float32))
    b_value = np.asarray(inputs["b_value"], dtype=np.float32)
    seed = np.asarray(inputs["seed"], dtype=np.float32)

    N, D = x.shape
    _, H, C = seed.shape
    HC = H * C
    M = 8
    host_alpha = os.environ.get("KERNEL_HOST_ALPHA", "0") == "1"

    # ---- edges: pure relabel on host (no efficient device gather) ----
    e = batch[rei]
    new_edge_index = np.concatenate([e, e[::-1]], axis=1).astype(rei.dtype)

    # ---- fold seed into key projection ----
    q = seed[0]                                      # [H, C]
    Wq = np.ascontiguousarray(
        (W_key.reshape(D, H, C) * q[None]).sum(-1)
    )                                                # [D, H]
    bq = (b_key.reshape(H, C) * q).sum(-1)           # [H]
    kfac = np.exp(bq / math.sqrt(C)).astype(np.float64)

    # ---- shard nodes at segment boundaries ----
    splits = [0]
    for i in range(1, M):
        tgt = (N * i) // M
        splits.append(int(np.searchsorted(batch, batch[tgt])))
    splits.append(N)

    core_meta = []
    nsup_max, span_max = 1, 1
    for i in range(M):
        a, b = splits[i], splits[i + 1]
        nloc = b - a
        if nloc == 0:
            core_meta.append((a, b, 0, 0, np.zeros(0, np.int64), 0))
            continue
        seg_lo = int(batch[a])
        seg_hi = int(batch[b - 1]) + 1
        bloc = (batch[a:b] - seg_lo).astype(np.int64)
        nsup = (nloc + ST - 1) // ST
        stbase = bloc[np.arange(nsup) * ST]
        ends = np.minimum((np.arange(nsup) + 1) * ST, nloc) - 1
        span = int((bloc[ends] - stbase).max()) + 1
        core_meta.append((a, b, seg_lo, seg_hi, stbase, nsup))
        nsup_max = max(nsup_max, nsup)
        span_max = max(span_max, span)

    WG = max(8, ((span_max + 7) // 8) * 8)
    NSUP = nsup_max
    AUXW = WG + 4 + (H if host_alpha else 0)

    if host_alpha:
        araw = (x @ Wq) / math.sqrt(C)
        araw_mx = araw.max(axis=0)
        ex_all = np.exp(araw - araw_mx[None, :]).astype(np.float32)
        kfac = kfac * np.exp(araw_mx.astype(np.float64))

    # ---- per-core padded slabs in DMA-friendly [t, p, tt, :] layout ----
    in_maps = []
    for i in range(M):
        a, b, seg_lo, seg_hi, stbase, nsup = core_meta[i]
        nloc = b - a
        x_sl = np.zeros((NSUP * ST, D), np.float32)
        x_sl[:nloc] = x[a:b]
        aux = np.zeros((NSUP * ST, AUXW), np.float32)
        if nloc:
            bloc = (batch[a:b] - seg_lo).astype(np.int64)
            stb_node = np.repeat(stbase, ST)[:nloc]
            w_idx = bloc - stb_node
            assert w_idx.min() >= 0 and w_idx.max() < WG
            aux[np.arange(nloc), w_idx] = 1.0
            aux[:nloc, WG : WG + 3] = position[a:b]
            aux[:nloc, WG + 3] = 1.0
            if host_alpha:
                aux[:nloc, WG + 4 :] = ex_all[a:b]

        def interleave(arr, width):
            return np.ascontiguousarray(
                arr.reshape(NSUP, ST_TILES, P, width)
                .transpose(0, 2, 1, 3)
                .reshape(NSUP * P, ST_TILES * width)
            )

        im = {
            "x": interleave(x_sl, D),
            "aux": interleave(aux, AUXW),
            "wv": W_value,
        }
        if not host_alpha:
            im["wq"] = Wq
        in_maps.append(im)

    # ---- build + run ----
    if os.environ.get("KERNEL_SIM", "0") == "1":
        res = _numpy_sim(in_maps, NSUP, WG, D, H, C, host_alpha, kfac)
    else:
        key = (NSUP, WG, D, H, C, host_alpha, tuple(np.round(kfac, 12)))
        if key not in _GRAPH_CACHE:
            _GRAPH_CACHE.clear()
            _GRAPH_CACHE[key] = _build_graph(NSUP, WG, D, H, C, host_alpha, kfac)
        nc = _GRAPH_CACHE[key]

        from concourse.bass_utils import run_bass_kernel_spmd

        trace = os.environ.get("KERNEL_TRACE", "0") == "1"
        if trace:
            try:
                _ensure_ntff_hook()
            except Exception:
                trace = False
        res = run_bass_kernel_spmd(nc, in_maps, core_ids=list(range(M)), trace=trace)
    LAST_RESULT = res
    LAST_EXEC_NS = getattr(res, "exec_time_ns", None)

    # ---- host merge epilogue (tiny: [B, *] only) ----
    out_full = np.zeros((B, HC), np.float32)
    npos_full = np.zeros((B, 3), np.float32)
    for i in range(M):
        a, b, seg_lo, seg_hi, stbase, nsup = core_meta[i]
        if b - a == 0:
            continue
        S = seg_hi - seg_lo
        proj = res.results[i]["proj"].reshape(NSUP, C, H, WG)
        pd = res.results[i]["pd"].reshape(NSUP, 4, WG + H * WG)
        outpre = np.zeros((S + WG, HC), np.float64)
        possum = np.zeros((S + WG, 3), np.float64)
        counts = np.zeros(S + WG, np.float64)
        denom = np.zeros((S + WG, H), np.float64)
        for t in range(nsup):
            b0 = int(stbase[t])
            sl = slice(b0, b0 + WG)
            outpre[sl] += proj[t].transpose(2, 1, 0).reshape(WG, HC)
            possum[sl] += pd[t, :3, :WG].T
            counts[sl] += pd[t, 3, :WG]
            denom[sl] += pd[t, 3, WG:].reshape(H, WG).T
        outpre, possum, counts, denom = (
            outpre[:S], possum[:S], counts[:S], denom[:S],
        )
        den_rep = np.repeat(denom, C, axis=1)
        out_i = (outpre + den_rep * b_value[None, :]) / np.clip(
            den_rep, 1e-16, None
        )
        npos_i = possum / np.clip(counts, 1.0, None)[:, None]
        out_full[seg_lo:seg_hi] = out_i.astype(np.float32)
        npos_full[seg_lo:seg_hi] = npos_i.astype(np.float32)

    return out_full, new_edge_index, npos_full


# revision 28
# speedup vs baseline: 3.4540x; 3.4540x over previous
"""
AdaptPooling (seed-attention segment softmax pooling) on 8 TRN2 NeuronCores.

Math (reference):
    key   = x @ W_key + b_key            [N, H, C]
    alpha = <key, seed> / sqrt(C)        [N, H]
    softmax of alpha within each segment (batch is SORTED)
    out   = segsum(alpha_n * (x @ W_value + b_value))    [B, H*C]
    new_position = segmean(position)                     [B, 3]
    new_edge_index = concat(batch[raw_edge_index], swap) [2, 2E]

Device strategy (data-parallel over nodes, split at segment boundaries so
segment reductions never cross cores -> no collectives needed):

  * alpha only touches x through Wq[d,h] = sum_c W_key[d,h*C+c]*seed[h,c],
    so alpha = (x @ Wq + bq)/sqrt(C): a [D,H] projection (host-folded).
  * softmax max-subtraction is dropped (shift invariant; scores are O(1));
    per-head constant exp(bq/sqrt(C)) folds into the one-hot build.
  * weighted aggregation uses the moment trick:
        G[d, h, s] = sum_n x[n,d] * ex[n,h] * onehot[n,s]
        out_pre[s, hc] = sum_d G[d,h,s] * W_value[d,hc]
    This contracts over NODES (natural x layout, no value materialization,
    x is read exactly once).
  * batch is an input -> host knows the segment structure and bakes a
    static windowed layout: nodes are packed into "super-tiles" of 1024;
    each super-tile's segments fit a window of WG segment slots. PSUM
    accumulates [128d x H*WG] over the super-tile; windows are staged to
    DRAM and the (tiny) overlap-add + divide epilogue runs on host.
  * edge remap is a pure relabel gather (batch[idx]); TRN2 has no
    efficient fine-grained gather (indirect-DMA is descriptor-bound at
    4B/descriptor), so it stays on host.
"""

import math
import os
import sys
from contextlib import ExitStack

import ml_dtypes
import numpy as np

sys.path.insert(0, "/opt/trn_rl_repo")

P = 128          # partitions
ST_TILES = 8     # 128-node tiles per super-tile
ST = P * ST_TILES

# set by kernel() after each run (read by test.py)
LAST_EXEC_NS = None
LAST_RESULT = None

_GRAPH_CACHE = {}


def _split_multi_waits(nc):
    """walrus on this toolchain accepts only ONE semaphore wait per
    instruction; split extra waits into preceding wait-only NoOps."""
    from concourse import mybir

    n = 0
    for fn in nc.m.functions:
        for bb in fn.blocks:
            new_list = []
            for inst in bb.instructions:
                si = inst.sync_info
                ow = list(si.on_wait) if si is not None and si.on_wait else []
                if len(ow) > 1:
                    for w in ow[:-1]:
                        nop = mybir.InstNoOp(
                            name=f"{inst.name}-wsplit{n}", ins=[], outs=[]
                        )
                        nop.engine = inst.engine
                        nop.sync_info = mybir.SyncInfo(on_wait=[w], on_update=[])
                        new_list.append(nop)
                        n += 1
                    si.on_wait = [ow[-1]]
                    inst.sync_info = si
                new_list.append(inst)
            bb.instructions = new_list
    return n


def _build_graph(NSUP, WG, D, H, C, host_alpha):
    from concourse import bass, mybir
    from concourse.tile import TileContext

    f32 = mybir.dt.float32
    bf16 = mybir.dt.bfloat16
    HC = H * C
    DC = D // P
    AUXW = WG + 4 + (H if host_alpha else 0)
    inv_sqrt_c = 1.0 / math.sqrt(C)

    nc = bass.Bass(target_bir_lowering=False)
    x_p = nc.declare_dram_parameter(
        "x", [NSUP * P, ST_TILES * D], bf16, isOutput=False
    )
    aux_p = nc.declare_dram_parameter(
        "aux", [NSUP * P, ST_TILES * AUXW], bf16, isOutput=False
    )
    wv_p = nc.declare_dram_parameter("wv", [D, HC], bf16, isOutput=False)
    if not host_alpha:
        wq_p = nc.declare_dram_parameter("wq", [D, H], bf16, isOutput=False)
    proj_p = nc.declare_dram_parameter("proj", [NSUP * C, H * WG], f32, isOutput=True)
    pd_p = nc.declare_dram_parameter("pd", [NSUP * 4, WG + H * WG], f32, isOutput=True)

    with TileContext(nc) as tc, ExitStack() as ctx:
        consts = ctx.enter_context(tc.tile_pool(name="consts", bufs=1))
        xin = ctx.enter_context(tc.tile_pool(name="xin", bufs=3))
        work = ctx.enter_context(tc.tile_pool(name="work", bufs=3))
        stg = ctx.enter_context(tc.tile_pool(name="stg", bufs=2))
        ps_g = ctx.enter_context(
            tc.tile_pool(name="ps_g", bufs=2 if host_alpha else 1, space="PSUM")
        )
        ps_s = ctx.enter_context(tc.tile_pool(name="ps_s", bufs=1, space="PSUM"))

        # weights, d-chunked to [128, DC, *]; laundered once via DVE
        wv_sb = consts.tile([P, DC, HC], bf16)
        nc.sync.dma_start(
            out=wv_sb[:], in_=wv_p[:].rearrange("(dc p) hc -> p dc hc", p=P)
        )
        if not host_alpha:
            wq_sb = consts.tile([P, DC, H], bf16)
            nc.sync.dma_start(
                out=wq_sb[:], in_=wq_p[:].rearrange("(dc p) h -> p dc h", p=P)
            )
            ident = consts.tile([P, P], bf16)
            nc.gpsimd.memset(ident[:], 0.0)
            nc.gpsimd.affine_select(
                out=ident[:],
                in_=ident[:],
                compare_op=mybir.AluOpType.not_equal,
                fill=1.0,
                base=0,
                pattern=[[-1, P]],
                channel_multiplier=1,
            )

        for t in range(NSUP):
            x_t = xin.tile([P, ST_TILES * D], bf16, tag="x")
            aux_t = xin.tile([P, ST_TILES * AUXW], bf16, tag="aux")
            nc.sync.dma_start(out=x_t[:], in_=x_p[t * P : (t + 1) * P, :])
            nc.sync.dma_start(out=aux_t[:], in_=aux_p[t * P : (t + 1) * P, :])

            g_ps = []
            for ck in range(DC):
                g_ck = ps_g.tile([P, H * WG], f32, tag=f"g{ck}", name=f"g{ck}_{t}")
                g_ps.append(g_ck)
            pos_ps = ps_s.tile([4, WG], f32, tag="pos")
            den_ps = ps_s.tile([4, H * WG], f32, tag="den")

            for tt in range(ST_TILES):
                xa = tt * D
                aa = tt * AUXW
                cmp_ap = aux_t[:, aa : aa + WG]
                posw_ap = aux_t[:, aa + WG : aa + WG + 4]

                if host_alpha:
                    ex_ap = aux_t[:, aa + WG + 4 : aa + WG + 4 + H]
                else:
                    xT_sb = []
                    for ck in range(DC):
                        xT_ps = ps_s.tile([P, P], f32, tag=f"xT{ck}")
                        nc.tensor.transpose(
                            out=xT_ps[:],
                            in_=x_t[:, xa + ck * P : xa + (ck + 1) * P],
                            identity=ident[:],
                        )
                        xs = work.tile([P, P], bf16, tag=f"xTs{ck}")
                        nc.vector.tensor_copy(out=xs[:], in_=xT_ps[:])
                        xT_sb.append(xs)
                    araw_ps = ps_s.tile([P, H], f32, tag="araw")
                    for ck in range(DC):
                        nc.tensor.matmul(
                            out=araw_ps[:],
                            lhsT=xT_sb[ck][:],
                            rhs=wq_sb[:, ck, :],
                            start=(ck == 0),
                            stop=(ck == DC - 1),
                        )
                    ex_sb = work.tile([P, H], bf16, tag="ex")
                    nc.scalar.activation(
                        out=ex_sb[:],
                        in_=araw_ps[:],
                        func=mybir.ActivationFunctionType.Exp,
                        scale=inv_sqrt_c,
                    )
                    ex_ap = ex_sb[:]

                oh_sb = work.tile([P, H * WG], bf16, tag="oh")
                nc.vector.tensor_tensor(
                    out=oh_sb[:].rearrange("p (h w) -> p h w", h=H),
                    in0=cmp_ap[:, None, :].to_broadcast([P, H, WG]),
                    in1=ex_ap[:, :, None].to_broadcast([P, H, WG]),
                    op=mybir.AluOpType.mult,
                )

                first = tt == 0
                last = tt == ST_TILES - 1
                for ck in range(DC):
                    nc.tensor.matmul(
                        out=g_ps[ck][:],
                        lhsT=x_t[:, xa + ck * P : xa + (ck + 1) * P],
                        rhs=oh_sb[:],
                        start=first,
                        stop=last,
                    )
                nc.tensor.matmul(
                    out=pos_ps[:], lhsT=posw_ap, rhs=cmp_ap, start=first, stop=last
                )
                nc.tensor.matmul(
                    out=den_ps[:], lhsT=posw_ap, rhs=oh_sb[:], start=first, stop=last
                )

            # G -> SBUF, project with W_value, stage out
            g_sb = stg.tile([P, DC, H * WG], bf16, tag="gsb")
            for ck in range(DC):
                nc.vector.tensor_copy(out=g_sb[:, ck, :], in_=g_ps[ck][:])
            proj_ps = ps_s.tile([C, H * WG], f32, tag="proj")
            for h in range(H):
                for ck in range(DC):
                    nc.tensor.matmul(
                        out=proj_ps[:, h * WG : (h + 1) * WG],
                        lhsT=wv_sb[:, ck, h * C : (h + 1) * C],
                        rhs=g_sb[:, ck, h * WG : (h + 1) * WG],
                        start=(ck == 0),
                        stop=(ck == DC - 1),
                    )
            proj_sb = stg.tile([C, H * WG], f32, tag="projsb")
            nc.vector.tensor_copy(out=proj_sb[:], in_=proj_ps[:])
            pd_sb = stg.tile([4, WG + H * WG], f32, tag="pdsb")
            nc.scalar.copy(out=pd_sb[:, :WG], in_=pos_ps[:])
            nc.scalar.copy(out=pd_sb[:, WG:], in_=den_ps[:])
            nc.sync.dma_start(out=proj_p[t * C : (t + 1) * C, :], in_=proj_sb[:])
            nc.sync.dma_start(out=pd_p[t * 4 : (t + 1) * 4, :], in_=pd_sb[:])

    _split_multi_waits(nc)
    return nc


def _ensure_ntff_hook():
    """The image's antenv lacks axon_hooks; provide it and register the
    ctypes NTFF profile hook so trace=True yields exec_time_ns."""
    import types

    try:
        from antenv import axon_hooks  # noqa: F401
    except ImportError:
        import antenv

        m = types.ModuleType("antenv.axon_hooks")
        m._hook = None
        m.set_axon_ntff_profile_hook = lambda h: setattr(m, "_hook", h)
        m.get_axon_ntff_profile_hook = lambda: getattr(m, "_hook", None)
        sys.modules["antenv.axon_hooks"] = m
        antenv.axon_hooks = m
    from antenv.axon_hooks import (
        get_axon_ntff_profile_hook,
        set_axon_ntff_profile_hook,
    )

    if get_axon_ntff_profile_hook() is None:
        from trn_agent_boot.trn_boot import _ntff_profile_via_ctypes

        set_axon_ntff_profile_hook(
            _ntff_profile_via_ctypes("/opt/axon/libaxon_pjrt.so")
        )


class _SimRes:
    def __init__(self, results):
        self.results = results
        self.exec_time_ns = None


def _numpy_sim(in_maps, NSUP, WG, D, H, C, host_alpha):
    """Bit-faithful numpy emulation of the device graph (for layout checks)."""
    AUXW = WG + 4 + (H if host_alpha else 0)
    results = []
    for im in in_maps:
        # undo the [t, p, tt, :] interleave back to node order
        x_sl = (
            im["x"].astype(np.float32)
            .reshape(NSUP, P, ST_TILES, D).transpose(0, 2, 1, 3).reshape(-1, D)
        )
        aux = (
            im["aux"].astype(np.float32)
            .reshape(NSUP, P, ST_TILES, AUXW)
            .transpose(0, 2, 1, 3)
            .reshape(-1, AUXW)
        )
        wv = im["wv"].astype(np.float32)
        cmp_ = aux[:, :WG]
        posw = aux[:, WG : WG + 4]
        if host_alpha:
            ex = aux[:, WG + 4 : WG + 4 + H]
        else:
            ex = np.exp((x_sl @ im["wq"].astype(np.float32)) / math.sqrt(C))
        proj = np.zeros((NSUP * C, H * WG), np.float32)
        pd = np.zeros((NSUP * 4, WG + H * WG), np.float32)
        for t in range(NSUP):
            sl = slice(t * ST, (t + 1) * ST)
            oh = (ex[sl][:, :, None] * cmp_[sl][:, None, :]).reshape(ST, H * WG)
            G = x_sl[sl].T @ oh                            # [D, H*WG]
            pos = posw[sl].T @ cmp_[sl]                    # [4, WG]
            den = posw[sl].T @ oh                          # [4, H*WG]
            pr = np.zeros((C, H * WG), np.float32)
            for h in range(H):
                pr[:, h * WG : (h + 1) * WG] = (
                    wv[:, h * C : (h + 1) * C].T @ G[:, h * WG : (h + 1) * WG]
                )
            proj[t * C : (t + 1) * C] = pr
            pd[t * 4 : (t + 1) * 4, :WG] = pos
            pd[t * 4 : (t + 1) * 4, WG:] = den
        results.append({"proj": proj, "pd": pd})
    return _SimRes(results)


def kernel(**inputs):
    global LAST_EXEC_NS, LAST_RESULT
    x = np.ascontiguousarray(np.asarray(inputs["x"], dtype=np.float32))
    position = np.ascontiguousarray(np.asarray(inputs["position"], dtype=np.float32))
    batch = np.ascontiguousarray(np.asarray(inputs["batch"], dtype=np.int32))
    rei = np.asarray(inputs["raw_edge_index"])
    B = int(np.asarray(inputs["num_segments"]))
    W_key = np.asarray(inputs["W_key"], dtype=np.float32)
    b_key = np.asarray(inputs["b_key"], dtype=np.float32)
    W_value = np.ascontiguousarray(np.asarray(inputs["W_value"], dtype=np.float32))
    b_value = np.asarray(inputs["b_value"], dtype=np.float32)
    seed = np.asarray(inputs["seed"], dtype=np.float32)

    N, D = x.shape
    _, H, C = seed.shape
    HC = H * C
    M = 8
    host_alpha = os.environ.get("KERNEL_HOST_ALPHA", "0") == "1"

    # ---- edges: pure relabel on host (no efficient device gather) ----
    e = batch[rei]
    new_edge_index = np.concatenate([e, e[::-1]], axis=1).astype(rei.dtype)

    # ---- fold seed into key projection ----
    q = seed[0]                                      # [H, C]
    Wq = np.ascontiguousarray(
        (W_key.reshape(D, H, C) * q[None]).sum(-1)
    )                                                # [D, H]
    # per-head constant factors (exp of bq, global alpha max shift) cancel
    # in the softmax ratio, so neither bq nor a max-shift changes `out`.

    # ---- shard nodes at segment boundaries ----
    splits = [0]
    for i in range(1, M):
        tgt = (N * i) // M
        splits.append(int(np.searchsorted(batch, batch[tgt])))
    splits.append(N)

    core_meta = []
    nsup_max, span_max = 1, 1
    for i in range(M):
        a, b = splits[i], splits[i + 1]
        nloc = b - a
        if nloc == 0:
            core_meta.append((a, b, 0, 0, np.zeros(0, np.int64), 0))
            continue
        seg_lo = int(batch[a])
        seg_hi = int(batch[b - 1]) + 1
        bloc = (batch[a:b] - seg_lo).astype(np.int64)
        nsup = (nloc + ST - 1) // ST
        stbase = bloc[np.arange(nsup) * ST]
        ends = np.minimum((np.arange(nsup) + 1) * ST, nloc) - 1
        span = int((bloc[ends] - stbase).max()) + 1
        core_meta.append((a, b, seg_lo, seg_hi, stbase, nsup))
        nsup_max = max(nsup_max, nsup)
        span_max = max(span_max, span)

    WG = max(8, ((span_max + 7) // 8) * 8)
    NSUP = nsup_max
    AUXW = WG + 4 + (H if host_alpha else 0)

    if host_alpha:
        araw = (x @ Wq) / math.sqrt(C)
        ex_all = np.exp(araw - araw.max(axis=0)[None, :]).astype(np.float32)

    # ---- per-core padded slabs in DMA-friendly [t, p, tt, :] layout ----
    in_maps = []
    for i in range(M):
        a, b, seg_lo, seg_hi, stbase, nsup = core_meta[i]
        nloc = b - a
        x_sl = np.zeros((NSUP * ST, D), np.float32)
        x_sl[:nloc] = x[a:b]
        aux = np.zeros((NSUP * ST, AUXW), np.float32)
        if nloc:
            bloc = (batch[a:b] - seg_lo).astype(np.int64)
            stb_node = np.repeat(stbase, ST)[:nloc]
            w_idx = bloc - stb_node
            assert w_idx.min() >= 0 and w_idx.max() < WG
            aux[np.arange(nloc), w_idx] = 1.0
            aux[:nloc, WG : WG + 3] = position[a:b]
            aux[:nloc, WG + 3] = 1.0
            if host_alpha:
                aux[:nloc, WG + 4 :] = ex_all[a:b]

        def interleave(arr, width):
            return np.ascontiguousarray(
                arr.reshape(NSUP, ST_TILES, P, width)
                .transpose(0, 2, 1, 3)
                .reshape(NSUP * P, ST_TILES * width)
                .astype(ml_dtypes.bfloat16)
            )

        im = {
            "x": interleave(x_sl, D),
            "aux": interleave(aux, AUXW),
            "wv": W_value.astype(ml_dtypes.bfloat16),
        }
        if not host_alpha:
            im["wq"] = Wq.astype(ml_dtypes.bfloat16)
        in_maps.append(im)

    # ---- build + run ----
    if os.environ.get("KERNEL_SIM", "0") == "1":
        res = _numpy_sim(in_maps, NSUP, WG, D, H, C, host_alpha)
    else:
        key = (NSUP, WG, D, H, C, host_alpha)
        if key not in _GRAPH_CACHE:
            _GRAPH_CACHE.clear()
            _GRAPH_CACHE[key] = _build_graph(NSUP, WG, D, H, C, host_alpha)
        nc = _GRAPH_CACHE[key]

        from concourse.bass_utils import run_bass_kernel_spmd

        trace = os.environ.get("KERNEL_TRACE", "0") == "1"
        if trace:
            try:
                _ensure_ntff_hook()
            except Exception:
                trace = False
        res = run_bass_kernel_spmd(nc, in_maps, core_ids=list(range(M)), trace=trace)
    LAST_RESULT = res
    LAST_EXEC_NS = getattr(res, "exec_time_ns", None)

    # ---- host merge epilogue (tiny: [B, *] only) ----
    out_full = np.zeros((B, HC), np.float32)
    npos_full = np.zeros((B, 3), np.float32)
    for i in range(M):
        a, b, seg_lo, seg_hi, stbase, nsup = core_meta[i]
        if b - a == 0:
            continue
        S = seg_hi - seg_lo
        proj = res.results[i]["proj"].reshape(NSUP, C, H, WG)
        pd = res.results[i]["pd"].reshape(NSUP, 4, WG + H * WG)
        outpre = np.zeros((S + WG, HC), np.float64)
        possum = np.zeros((S + WG, 3), np.float64)
        counts = np.zeros(S + WG, np.float64)
        denom = np.zeros((S + WG, H), np.float64)
        for t in range(nsup):
            b0 = int(stbase[t])
            sl = slice(b0, b0 + WG)
            outpre[sl] += proj[t].transpose(2, 1, 0).reshape(WG, HC)
            possum[sl] += pd[t, :3, :WG].T
            counts[sl] += pd[t, 3, :WG]
            denom[sl] += pd[t, 3, WG:].reshape(H, WG).T
        outpre, possum, counts, denom = (
            outpre[:S], possum[:S], counts[:S], denom[:S],
        )
        den_rep = np.repeat(denom, C, axis=1)
        out_i = (outpre + den_rep * b_value[None, :]) / np.clip(
            den_rep, 1e-16, None
        )
        npos_i = possum / np.clip(counts, 1.0, None)[:, None]
        out_full[seg_lo:seg_hi] = out_i.astype(np.float32)
        npos_full[seg_lo:seg_hi] = npos_i.astype(np.float32)

    return out_full, new_edge_index, npos_full
